# revision 31
# baseline (speedup 1.0000x reference)
"""TRN2 Bass kernel for nn_AttentionMP (GNN message passing attention).

Row-parallel attention across 8 NeuronCores: core c owns query rows
[c*1024, (c+1)*1024). Scores are computed TRANSPOSED, sT[j, i] (j = key
index on partitions, i = this core's query rows on the free dim), which
makes att^T directly available as the moving operand of downstream
matmuls — no on-device transposes in the hot path. The tiny
data-independent products fold on the host: qk = (Wq Wk^T)^T Hq^T ships
per-core (so no on-device q/k projection at all), W1v = Wv @ W1 ships
replicated (eliminating the v projection together with the Z
reassociation below).

Masking is split across engines to balance the pipeline (the PE would
otherwise be the bottleneck at 3 matmul-passes/tile):
 - j-tiles with (jt % 8) in {1,4,7} (includes tile 63): additive mask on
   the PE — adj ships as fp8 and lands in the scores PSUM as 240*adj via
   an identity matmul (lhsT = 240*I fp8); ACT computes exp(s + 240*m - 270):
   exp(s-30) unmasked, 0.0 exactly for masked entries.
 - the rest (5/8, includes tile 0 so the first exp needs no mask matmul
   and tile 63 never needs the DVE mult): multiplicative mask on the DVE
   — scores skip the
   mask matmul, ACT computes exp(s - 30) unmasked (bf16, finite), then
   e *= adj with adj shipped as bf16 (0/1 exact, all-bf16 tensor_mult
   runs in 2x_1p mode).
The -30 is a global stabilizer that cancels in normalization.

e is bf16: the softmax accumulator runs on DVE in bf16 (2x_1p) and
att@v is reassociated into Z[c,i] = sum_j H[j,c] e[j,i], accumulated in
PSUM across j-tiles with bf16 H as lhsT (natural-layout pretiled HN
chunks). PSUM: one triple-buffered pool of [128,1024] tiles (6 banks) +
the Z accumulator (2 banks); small stage-2 outputs use slices of the big
pool tiles. Because relu commutes with positive per-row scaling, softmax
normalization is deferred through the whole MLP:
    out = relu(relu(U@W1 + d*b1)@W2 + d*b2) / d,   U = Z^T @ Wv
so the MLP runs transposed with stationary weights and d*b enters via
rank-1 matmuls (den = ones @ (acc + e_last): the last tile's e never
touches the DVE accumulator — it rides a second accumulating matmul).
The final layer is re-flipped: per 128-row block, lhsT = hidden block
(SBUF) x W2 puts the output non-transposed in PSUM, and relu with the
per-row 1/d scale (ACT/DVE alternating) writes the staging tile
directly — no transposes and no extra PSUM->SBUF copy in the tail.
"""
import numpy as np
import ml_dtypes
import concourse.bass as bass
from concourse import bacc
import concourse.mybir as mybir
from concourse.tile import TileContext
from concourse.bass_utils import run_bass_kernel_spmd

N = 8192
D = 128
NC = 8
RPC = N // NC          # rows per core = 1024
JT = N // 128          # j tiles = 64
F32 = mybir.dt.float32
F32R = mybir.dt.float32r
BF16 = mybir.dt.bfloat16
FP8 = mybir.dt.float8e4
MASK_D = 240.0         # fp8e4 max finite
STAB = 30.0            # global score shift, cancels in softmax
HT_CHUNKS = 4
GRP = 8                # j-tiles per adj DMA batch group

PE_MASKED = ([5, 7] + [8 + r for r in (1, 3, 5, 7)]
             + [8 * g + r for g in range(2, 8) for r in (1, 4, 7)])
DVE_MASKED = [jt for jt in range(JT) if jt not in PE_MASKED]
NE = len(PE_MASKED)    # 24
NO = len(DVE_MASKED)   # 40
E_IDX = {jt: i for i, jt in enumerate(PE_MASKED)}
O_IDX = {jt: i for i, jt in enumerate(DVE_MASKED)}
E_CNT = [sum(1 for jt in PE_MASKED if jt // 8 == g) for g in range(8)]
O_CNT = [sum(1 for jt in DVE_MASKED if jt // 8 == g) for g in range(8)]
E_BASE = [sum(E_CNT[:g]) for g in range(8)]
O_BASE = [sum(O_CNT[:g]) for g in range(8)]

_CACHED = {}


def build(with_bias=False):
    nc = bacc.Bacc("TRN2", target_bir_lowering=False, debug=True)

    HTC = [nc.dram_tensor(f"HT{t}", [D, N // HT_CHUNKS], F32R, kind="ExternalInput")
           for t in range(HT_CHUNKS)]
    HNC = [nc.dram_tensor(f"HN{t}", [D, N // 4], BF16, kind="ExternalInput")
           for t in range(4)]  # pretiled [p, t*128+c], bf16
    QK = nc.dram_tensor("QK", [D, RPC], F32R, kind="ExternalInput")
    ADJE = nc.dram_tensor("ADJE", [NE * 128, RPC], FP8, kind="ExternalInput")
    ADJO = nc.dram_tensor("ADJO", [NO * 128, RPC], BF16, kind="ExternalInput")
    W1V = nc.dram_tensor("W1V", [D, D], F32R, kind="ExternalInput")
    W2 = nc.dram_tensor("W2", [D, D], F32R, kind="ExternalInput")
    B1R = nc.dram_tensor("B1R", [1, D], F32R, kind="ExternalInput")
    B2R = nc.dram_tensor("B2R", [1, D], F32R, kind="ExternalInput")
    I240 = nc.dram_tensor("I240", [D, D], FP8, kind="ExternalInput")
    ONES = nc.dram_tensor("ONES", [D, D], BF16, kind="ExternalInput")
    IDENT = nc.dram_tensor("IDENT", [1, 1], F32, kind="ExternalInput")
    BIASE = nc.dram_tensor("BIASE", [D, 1], F32, kind="ExternalInput")
    BIASO = nc.dram_tensor("BIASO", [D, 1], F32, kind="ExternalInput")
    OUT = nc.dram_tensor("OUT", [RPC, D], F32, kind="ExternalOutput")

    adjE_rows = ADJE.rearrange("(t p) i -> p t i", p=128)  # [p, E-tile, i]
    adjO_rows = ADJO.rearrange("(t p) i -> p t i", p=128)  # [p, O-tile, i]

    # group g covers j-tiles [g*8, g*8+8): 3 PE-masked + 5 DVE-masked
    NG = JT // GRP

    with TileContext(nc) as tc:
        with (
            tc.tile_pool(name="pers", bufs=1) as pers,
            tc.tile_pool(name="adjE", bufs=4) as adjEp,
            tc.tile_pool(name="adjO", bufs=4) as adjOp,
            tc.tile_pool(name="ep", bufs=5) as ep,
            tc.tile_pool(name="psA", bufs=3, space="PSUM") as psA,   # 3x[128,1024]
            tc.tile_pool(name="psZ", bufs=1, space="PSUM") as psZ,   # Z accumulator
        ):
            # ---- persistent tiles
            htc = []
            for t in range(HT_CHUNKS):
                htc_t = pers.tile([D, N // HT_CHUNKS], F32R, tag=f"ht{t}")
                htc.append(htc_t)
            hnc = []
            for t in range(4):
                hnc_t = pers.tile([D, N // 4], BF16, tag=f"hn{t}")
                hnc.append(hnc_t)
            qk = pers.tile([D, RPC], F32R, tag="qk")
            w1v = pers.tile([D, D], F32R, tag="w1v")
            w2 = pers.tile([D, D], F32R, tag="w2")
            b1r = pers.tile([1, D], F32R, tag="b1r")
            b2r = pers.tile([1, D], F32R, tag="b2r")
            i240 = pers.tile([D, D], FP8, tag="i240")
            ones = pers.tile([D, D], BF16, tag="ones")
            ident = pers.tile([1, 1], F32, tag="ident")
            biase = pers.tile([D, 1], F32, tag="biase")
            biaso = pers.tile([D, 1], F32, tag="biaso")

            def e_group_tile(g):
                return adjEp.tile([128, 4 * RPC], FP8, tag="ae", name=f"ae{g}")

            def o_group_tile(g):
                return adjOp.tile([128, 6 * RPC], BF16, tag="ao", name=f"ao{g}")

            def e_base(g):  # first ADJE tile index of group g
                return E_BASE[g]

            def o_base(g):
                return O_BASE[g]

            # critical-path DMAs on the sync queue, most-urgent first
            # (packets drain roughly in issue order); bulk/late tensors go
            # on gpsimd's software queue.
            aE0 = e_group_tile(0)
            aO0 = o_group_tile(0)
            aE1 = e_group_tile(1)
            aO1 = o_group_tile(1)
            aE = {0: aE0, 1: aE1}
            aO = {0: aO0, 1: aO1}

            def adj_tile_dma(jt):
                """Issue the DMA for j-tile jt's adj slice (per-tile grain)."""
                g = jt // GRP
                if jt in E_IDX:
                    half = E_IDX[jt] - e_base(g)
                    nc.sync.dma_start(out=aE[g][:, half * RPC:(half + 1) * RPC],
                                      in_=adjE_rows[:, E_IDX[jt]])
                else:
                    half = O_IDX[jt] - o_base(g)
                    nc.sync.dma_start(out=aO[g][:, half * RPC:(half + 1) * RPC],
                                      in_=adjO_rows[:, O_IDX[jt]])

            nc.sync.dma_start(out=qk[:, 0:512], in_=QK[:, 0:512])
            nc.sync.dma_start(out=biaso[:], in_=BIASO[:])
            nc.sync.dma_start(out=htc[0][:, 0:256], in_=HTC[0][:, 0:256])
            nc.sync.dma_start(out=qk[:, 512:1024], in_=QK[:, 512:1024])
            nc.sync.dma_start(out=htc[0][:, 256:768], in_=HTC[0][:, 256:768])
            adj_tile_dma(0)
            adj_tile_dma(1)
            nc.sync.dma_start(out=i240[:], in_=I240[:])
            adj_tile_dma(2)
            adj_tile_dma(3)
            nc.sync.dma_start(out=htc[0][:, 768:N // HT_CHUNKS],
                              in_=HTC[0][:, 768:N // HT_CHUNKS])
            adj_tile_dma(4)
            # group-0 PE tiles (jt 5,7) in one grouped issue, then groups
            # 1-2 as grouped issues — fewer serial SP slots so the loop's
            # per-tile prefetch starts sooner.
            nc.sync.dma_start(out=aE0[:, 0:E_CNT[0] * RPC].rearrange(
                "p (k i) -> p k i", k=E_CNT[0]),
                in_=adjE_rows[:, 0:E_CNT[0]])
            nc.sync.dma_start(out=biase[:], in_=BIASE[:])
            adj_tile_dma(6)
            for gg in (1, 2):
                if gg == 2:
                    aE[2] = e_group_tile(2)
                    aO[2] = o_group_tile(2)
                nc.sync.dma_start(
                    out=aE[gg][:, 0:E_CNT[gg] * RPC].rearrange(
                        "p (k i) -> p k i", k=E_CNT[gg]),
                    in_=adjE_rows[:, E_BASE[gg]:E_BASE[gg] + E_CNT[gg]])
                nc.sync.dma_start(
                    out=aO[gg][:, 0:O_CNT[gg] * RPC].rearrange(
                        "p (k i) -> p k i", k=O_CNT[gg]),
                    in_=adjO_rows[:, O_BASE[gg]:O_BASE[gg] + O_CNT[gg]])
            for t, src in [(ident, IDENT), (w1v, W1V), (w2, W2),
                           (b1r, B1R), (b2r, B2R), (ones, ONES)]:
                nc.gpsimd.dma_start(out=t[:], in_=src[:])
            nc.gpsimd.dma_start(out=hnc[0][:], in_=HNC[0][:])
            nc.gpsimd.dma_start(out=hnc[1][:], in_=HNC[1][:])

            acc = pers.tile([D, RPC], BF16, tag="acc")
            zsb = pers.tile([D, RPC], F32R, tag="zsb")
            hts = pers.tile([D, RPC], F32R, tag="hts")    # hidden^T (SBUF)
            dentr = pers.tile([1, RPC], F32R, tag="dentr")
            dcol = pers.tile([D, NC], F32, tag="dcol")
            rcol = pers.tile([D, NC], F32, tag="rcol")
            outsb = pers.tile([D, NC * D], F32, tag="outsb")

            # ---- stage 1 (Z matmuls lag two j-tiles so scores(jt+1)
            # issue while exp(jt) runs)
            zps = psZ.tile([D, RPC], F32, tag="z")
            etiles = {}

            def do_z(jt):
                e_prev = etiles.pop(jt)
                htile = hnc[jt // 16][:, (jt % 16) * 128:(jt % 16 + 1) * 128]
                for h in range(2):
                    cs = slice(h * 512, (h + 1) * 512)
                    nc.tensor.matmul(zps[:, cs], lhsT=htile, rhs=e_prev[:, cs],
                                     start=(jt == 0), stop=(jt == JT - 1))

            cwq = N // HT_CHUNKS // 128
            for jt in range(JT):
                g, r = divmod(jt, GRP)
                # per-tile prefetch 2 groups ahead keeps the wires smooth:
                # one adj tile per iteration, H chunks in halves mid-group.
                tgt = jt + 3 * GRP
                if tgt < JT:
                    if tgt % GRP == 0:
                        gg = tgt // GRP
                        aE[gg] = e_group_tile(gg)
                        aO[gg] = o_group_tile(gg)
                    adj_tile_dma(tgt)
                hchalf = N // HT_CHUNKS // 2
                for gc in (1, 2, 3):
                    if jt == 16 * gc - 14:
                        nc.sync.dma_start(out=htc[gc][:, 0:hchalf],
                                          in_=HTC[gc][:, 0:hchalf])
                    elif jt == 16 * gc - 10:
                        nc.sync.dma_start(out=htc[gc][:, hchalf:],
                                          in_=HTC[gc][:, hchalf:])
                for gc in (2, 3):
                    if jt == 16 * gc - 6:
                        nc.sync.dma_start(out=hnc[gc][:], in_=HNC[gc][:])
                even = jt in E_IDX
                sps = psA.tile([D, RPC], F32, tag="big")
                ktile = htc[jt // cwq][:, (jt % cwq) * 128:(jt % cwq + 1) * 128]
                if even:
                    half = E_IDX[jt] - e_base(g)
                    adj_sb = aE[g]
                    for h in range(2):
                        cs = slice(h * 512, (h + 1) * 512)
                        nc.tensor.matmul(sps[:, cs], lhsT=i240[:],
                                         rhs=adj_sb[:, half * RPC + h * 512:
                                                    half * RPC + (h + 1) * 512],
                                         start=True, stop=False)
                        nc.tensor.matmul(sps[:, cs], lhsT=ktile, rhs=qk[:, cs],
                                         start=False, stop=True)
                else:
                    for h in range(2):
                        cs = slice(h * 512, (h + 1) * 512)
                        nc.tensor.matmul(sps[:, cs], lhsT=ktile, rhs=qk[:, cs],
                                         start=True, stop=True)
                e = ep.tile([D, RPC], BF16, tag="e")
                nc.scalar.activation(e[:], sps[:],
                                     mybir.ActivationFunctionType.Exp,
                                     bias=(biase[:] if even else biaso[:]))
                etiles[jt] = e
                if not even:
                    half = O_IDX[jt] - o_base(g)
                    adjo_sb = aO[g]
                    nc.vector.tensor_mul(e[:], e[:],
                                         adjo_sb[:, half * RPC:(half + 1) * RPC])
                if jt == 0:
                    nc.vector.tensor_copy(acc[:], e[:])
                elif jt < JT - 1:
                    nc.vector.tensor_add(acc[:], acc[:], e[:])
                if jt >= 2:
                    do_z(jt - 2)
            do_z(JT - 2)
            e_last = etiles[JT - 1]

            # ---- stage 2: denominators + normalization-deferred transposed MLP
            # den = ones @ (acc + e_last): e(63)'s DVE add is skipped — it
            # rides a second accumulating matmul instead.
            dps = psA.tile([D, RPC], F32, tag="big")
            for h in range(2):
                cs = slice(h * 512, (h + 1) * 512)
                nc.tensor.matmul(dps[:, cs], lhsT=ones[:], rhs=acc[:, cs],
                                 start=True, stop=False)
                nc.tensor.matmul(dps[:, cs], lhsT=ones[:], rhs=e_last[:, cs],
                                 start=False, stop=True)
            do_z(JT - 1)
            nc.vector.tensor_copy(dentr[:, 0:512], dps[0:1, 0:512])
            nc.vector.tensor_copy(dentr[:, 512:1024], dps[0:1, 512:1024])
            nc.scalar.copy(zsb[:, 0:512], zps[:, 0:512])
            nc.vector.tensor_copy(zsb[:, 512:1024], zps[:, 512:1024])
            # 1/denom columns via tiny PE transposes of the den row
            rps = psA.tile([D, RPC], F32, tag="big")
            for it in range(4):
                nc.tensor.transpose(rps[:, it:it + 1],
                                    dentr[0:1, it * 128:(it + 1) * 128].bitcast(F32),
                                    ident[0:1, 0:1])
            gps = psA.tile([D, RPC], F32, tag="big")
            for h in range(2):
                cs = slice(h * 512, (h + 1) * 512)
                nc.tensor.matmul(gps[:, cs], lhsT=w1v[:], rhs=zsb[:, cs],
                                 start=True, stop=not with_bias)
                if with_bias:
                    nc.tensor.matmul(gps[:, cs], lhsT=b1r[:], rhs=dentr[:, cs],
                                     start=False, stop=True)
            for it in range(4, NC):
                nc.tensor.transpose(rps[:, it:it + 1],
                                    dentr[0:1, it * 128:(it + 1) * 128].bitcast(F32),
                                    ident[0:1, 0:1])
            nc.scalar.activation(hts[:, 0:512], gps[:, 0:512],
                                 mybir.ActivationFunctionType.Relu)
            nc.vector.tensor_relu(hts[:, 512:1024], gps[:, 512:1024])
            nc.scalar.copy(dcol[:], rps[:, 0:NC])
            nc.vector.reciprocal(rcol[:], dcol[:])
            # final layer, re-flipped per 128-row block: lhsT = hts block so
            # the output block lands non-transposed in PSUM; relu + (1/d)
            # scale alternates ACT/DVE and writes the staging tile directly.
            outv = OUT.rearrange("(t p) d -> p t d", p=128)
            for it in range(NC):
                bps = psA.tile([D, RPC], F32, tag="big")
                nc.tensor.matmul(bps[:, 0:D], lhsT=hts[:, it * 128:(it + 1) * 128],
                                 rhs=w2[:], start=True, stop=not with_bias)
                if with_bias:
                    nc.tensor.matmul(bps[:, 0:D],
                                     lhsT=dentr[0:1, it * 128:(it + 1) * 128],
                                     rhs=b2r[:], start=False, stop=True)
                ob = outsb[:, it * 128:(it + 1) * 128]
                if it % 2 == 0:
                    nc.scalar.activation(ob, bps[:, 0:D],
                                         mybir.ActivationFunctionType.Relu,
                                         scale=rcol[:, it:it + 1])
                else:
                    nc.vector.tensor_scalar(ob, bps[:, 0:D], rcol[:, it:it + 1],
                                            0.0, op0=mybir.AluOpType.mult,
                                            op1=mybir.AluOpType.max)
                if it % 2 == 1:
                    nc.sync.dma_start(
                        out=outv[:, it - 1:it + 1],
                        in_=outsb[:, (it - 1) * D:(it + 1) * D].rearrange(
                            "p (t d) -> p t d", t=2))
    nc.finalize()
    return nc


def _prep(H, adj, Wq, Wk, Wv, W1, b1, W2, b2):
    f8 = ml_dtypes.float8_e4m3
    bf = ml_dtypes.bfloat16
    H32 = np.asarray(H, dtype=np.float32)
    HT = np.ascontiguousarray(H32.T)
    adj = np.asarray(adj)
    M = (np.asarray(Wq, np.float32) @ np.asarray(Wk, np.float32).T)
    base = {
        "W1V": np.ascontiguousarray(np.asarray(Wv, np.float32) @ np.asarray(W1, np.float32)),
        "W2": np.asarray(W2, np.float32),
        "B1R": np.asarray(b1, np.float32).reshape(1, D),
        "B2R": np.asarray(b2, np.float32).reshape(1, D),
        "I240": (np.eye(D, dtype=np.float32) * MASK_D).astype(f8),
        "ONES": np.ones((D, D), bf),
        "IDENT": np.eye(1, dtype=np.float32),
        "BIASE": np.full((D, 1), -(MASK_D + STAB), np.float32),
        "BIASO": np.full((D, 1), -STAB, np.float32),
    }
    cw = N // HT_CHUNKS
    for t in range(HT_CHUNKS):
        base[f"HT{t}"] = np.ascontiguousarray(HT[:, t * cw:(t + 1) * cw])
    HNP = np.ascontiguousarray(
        H32.reshape(JT, 128, D).transpose(1, 0, 2).reshape(128, N)).astype(bf)
    for t in range(4):
        base[f"HN{t}"] = np.ascontiguousarray(HNP[:, t * (N // 4):(t + 1) * (N // 4)])
    in_maps = []
    for c in range(NC):
        m = dict(base)
        m["QK"] = np.ascontiguousarray(M.T @ HT[:, c * RPC:(c + 1) * RPC])
        adjT4 = np.ascontiguousarray(
            adj[c * RPC:(c + 1) * RPC, :].T).reshape(JT, 128, RPC)
        m["ADJE"] = np.ascontiguousarray(
            adjT4[PE_MASKED].reshape(NE * 128, RPC)).astype(np.float32).astype(f8)
        m["ADJO"] = np.ascontiguousarray(
            adjT4[DVE_MASKED].reshape(NO * 128, RPC)).astype(np.float32).astype(bf)
        in_maps.append(m)
    return in_maps


def kernel(H, adj, Wq, Wk, Wv, W1, b1, W2, b2):
    wb = bool(np.any(np.asarray(b1)) or np.any(np.asarray(b2)))
    key = f"nc{int(wb)}"
    if key not in _CACHED:
        _CACHED[key] = build(with_bias=wb)
    in_maps = _prep(H, adj, Wq, Wk, Wv, W1, b1, W2, b2)
    res = run_bass_kernel_spmd(_CACHED[key], in_maps, list(range(NC)))
    return np.concatenate([res.results[c]["OUT"] for c in range(NC)], axis=0)


# revision 32
# speedup vs baseline: 1.1177x; 1.1177x over previous
"""TRN2 Bass kernel for nn_AttentionMP (GNN message passing attention).

Row-parallel attention across 8 NeuronCores: core c owns query rows
[c*1024, (c+1)*1024). Scores are computed TRANSPOSED, sT[j, i] (j = key
index on partitions, i = this core's query rows on the free dim), which
makes att^T directly available as the moving operand of downstream
matmuls — no on-device transposes in the hot path. The tiny
data-independent products fold on the host: qk = (Wq Wk^T)^T Hq^T ships
per-core (so no on-device q/k projection at all), W1v = Wv @ W1 ships
replicated (eliminating the v projection together with the Z
reassociation below).

Masking is split across engines to balance the pipeline (the PE would
otherwise be the bottleneck at 3 matmul-passes/tile):
 - j-tiles with (jt % 8) in {1,4,7} (includes tile 63): additive mask on
   the PE — adj ships as fp8 and lands in the scores PSUM as 240*adj via
   an identity matmul (lhsT = 240*I fp8); ACT computes exp(s + 240*m - 270):
   exp(s-30) unmasked, 0.0 exactly for masked entries.
 - the rest (5/8, includes tile 0 so the first exp needs no mask matmul
   and tile 63 never needs the DVE mult): multiplicative mask on the DVE
   — scores skip the
   mask matmul, ACT computes exp(s - 30) unmasked (bf16, finite), then
   e *= adj with adj shipped as bf16 (0/1 exact, all-bf16 tensor_mult
   runs in 2x_1p mode).
The -30 is a global stabilizer that cancels in normalization.

e is bf16: the softmax accumulator runs on DVE in bf16 (2x_1p) and
att@v is reassociated into Z[c,i] = sum_j H[j,c] e[j,i], accumulated in
PSUM across j-tiles with bf16 H as lhsT (natural-layout pretiled HN
chunks). PSUM: one triple-buffered pool of [128,1024] tiles (6 banks) +
the Z accumulator (2 banks); small stage-2 outputs use slices of the big
pool tiles. Because relu commutes with positive per-row scaling, softmax
normalization is deferred through the whole MLP:
    out = relu(relu(U@W1 + d*b1)@W2 + d*b2) / d,   U = Z^T @ Wv
so the MLP runs transposed with stationary weights and d*b enters via
rank-1 matmuls (den = ones @ (acc + e_last): the last tile's e never
touches the DVE accumulator — it rides a second accumulating matmul).
The final layer is re-flipped: per 128-row block, lhsT = hidden block
(SBUF) x W2 puts the output non-transposed in PSUM, and relu with the
per-row 1/d scale (ACT/DVE alternating) writes the staging tile
directly — no transposes and no extra PSUM->SBUF copy in the tail.
"""
import numpy as np
import ml_dtypes
import concourse.bass as bass
from concourse import bacc
import concourse.mybir as mybir
from concourse.tile import TileContext
from concourse.bass_utils import run_bass_kernel_spmd

N = 8192
D = 128
NC = 8
RPC = N // NC          # rows per core = 1024
JT = N // 128          # j tiles = 64
F32 = mybir.dt.float32
F32R = mybir.dt.float32r
BF16 = mybir.dt.bfloat16
FP8 = mybir.dt.float8e4
MASK_D = 240.0         # fp8e4 max finite
STAB = 30.0            # global score shift, cancels in softmax
HT_CHUNKS = 4
GRP = 8                # j-tiles per adj DMA batch group

PE_MASKED = ([5, 7] + [8 + r for r in (1, 3, 5, 7)]
             + [8 * g + r for g in range(2, 8) for r in (1, 4, 7)])
DVE_MASKED = [jt for jt in range(JT) if jt not in PE_MASKED]
NE = len(PE_MASKED)    # 24
NO = len(DVE_MASKED)   # 40
E_IDX = {jt: i for i, jt in enumerate(PE_MASKED)}
O_IDX = {jt: i for i, jt in enumerate(DVE_MASKED)}
E_CNT = [sum(1 for jt in PE_MASKED if jt // 8 == g) for g in range(8)]
O_CNT = [sum(1 for jt in DVE_MASKED if jt // 8 == g) for g in range(8)]
E_BASE = [sum(E_CNT[:g]) for g in range(8)]
O_BASE = [sum(O_CNT[:g]) for g in range(8)]

_CACHED = {}


def build(with_bias=False):
    nc = bacc.Bacc("TRN2", target_bir_lowering=False, debug=True)

    HTC = [nc.dram_tensor(f"HT{t}", [D, N // HT_CHUNKS], F32R, kind="ExternalInput")
           for t in range(HT_CHUNKS)]
    HNC = [nc.dram_tensor(f"HN{t}", [D, N // 4], BF16, kind="ExternalInput")
           for t in range(4)]  # pretiled [p, t*128+c], bf16
    QK = nc.dram_tensor("QK", [D, RPC], F32R, kind="ExternalInput")
    ADJE = nc.dram_tensor("ADJE", [NE * 128, RPC], FP8, kind="ExternalInput")
    ADJO = nc.dram_tensor("ADJO", [NO * 128, RPC], BF16, kind="ExternalInput")
    W1V = nc.dram_tensor("W1V", [D, D], F32R, kind="ExternalInput")
    W2 = nc.dram_tensor("W2", [D, D], F32R, kind="ExternalInput")
    B1R = nc.dram_tensor("B1R", [1, D], F32R, kind="ExternalInput")
    B2R = nc.dram_tensor("B2R", [1, D], F32R, kind="ExternalInput")
    I240 = nc.dram_tensor("I240", [D, D], FP8, kind="ExternalInput")
    ONES = nc.dram_tensor("ONES", [D, D], BF16, kind="ExternalInput")
    IDENT = nc.dram_tensor("IDENT", [1, 1], F32, kind="ExternalInput")
    BIASE = nc.dram_tensor("BIASE", [D, 1], F32, kind="ExternalInput")
    BIASO = nc.dram_tensor("BIASO", [D, 1], F32, kind="ExternalInput")
    OUT = nc.dram_tensor("OUT", [RPC, D], F32, kind="ExternalOutput")

    adjE_rows = ADJE.rearrange("(t p) i -> p t i", p=128)  # [p, E-tile, i]
    adjO_rows = ADJO.rearrange("(t p) i -> p t i", p=128)  # [p, O-tile, i]

    # group g covers j-tiles [g*8, g*8+8): 3 PE-masked + 5 DVE-masked
    NG = JT // GRP

    with TileContext(nc) as tc:
        with (
            tc.tile_pool(name="pers", bufs=1) as pers,
            tc.tile_pool(name="adjE", bufs=4) as adjEp,
            tc.tile_pool(name="adjO", bufs=4) as adjOp,
            tc.tile_pool(name="ep", bufs=5) as ep,
            tc.tile_pool(name="psA", bufs=3, space="PSUM") as psA,   # 3x[128,1024]
            tc.tile_pool(name="psZ", bufs=1, space="PSUM") as psZ,   # Z accumulator
        ):
            # ---- persistent tiles
            htc = []
            for t in range(HT_CHUNKS):
                htc_t = pers.tile([D, N // HT_CHUNKS], F32R, tag=f"ht{t}")
                htc.append(htc_t)
            hnc = []
            for t in range(4):
                hnc_t = pers.tile([D, N // 4], BF16, tag=f"hn{t}")
                hnc.append(hnc_t)
            qk = pers.tile([D, RPC], F32R, tag="qk")
            w1v = pers.tile([D, D], F32R, tag="w1v")
            w2 = pers.tile([D, D], F32R, tag="w2")
            b1r = pers.tile([1, D], F32R, tag="b1r")
            b2r = pers.tile([1, D], F32R, tag="b2r")
            i240 = pers.tile([D, D], FP8, tag="i240")
            ones = pers.tile([D, D], BF16, tag="ones")
            ident = pers.tile([1, 1], F32, tag="ident")
            biase = pers.tile([D, 1], F32, tag="biase")
            biaso = pers.tile([D, 1], F32, tag="biaso")

            def e_group_tile(g):
                return adjEp.tile([128, 4 * RPC], FP8, tag="ae", name=f"ae{g}")

            def o_group_tile(g):
                return adjOp.tile([128, 6 * RPC], BF16, tag="ao", name=f"ao{g}")

            def e_base(g):  # first ADJE tile index of group g
                return E_BASE[g]

            def o_base(g):
                return O_BASE[g]

            # critical-path DMAs on the sync queue, most-urgent first
            # (packets drain roughly in issue order); bulk/late tensors go
            # on gpsimd's software queue.
            aE0 = e_group_tile(0)
            aO0 = o_group_tile(0)
            aE1 = e_group_tile(1)
            aO1 = o_group_tile(1)
            aE = {0: aE0, 1: aE1}
            aO = {0: aO0, 1: aO1}

            def adj_tile_dma(jt):
                """Issue the DMA for j-tile jt's adj slice (per-tile grain)."""
                g = jt // GRP
                if jt in E_IDX:
                    half = E_IDX[jt] - e_base(g)
                    nc.sync.dma_start(out=aE[g][:, half * RPC:(half + 1) * RPC],
                                      in_=adjE_rows[:, E_IDX[jt]])
                else:
                    half = O_IDX[jt] - o_base(g)
                    nc.sync.dma_start(out=aO[g][:, half * RPC:(half + 1) * RPC],
                                      in_=adjO_rows[:, O_IDX[jt]])

            nc.sync.dma_start(out=qk[:, 0:512], in_=QK[:, 0:512])
            nc.sync.dma_start(out=biaso[:], in_=BIASO[:])
            nc.sync.dma_start(out=htc[0][:, 0:256], in_=HTC[0][:, 0:256])
            nc.sync.dma_start(out=qk[:, 512:1024], in_=QK[:, 512:1024])
            nc.sync.dma_start(out=htc[0][:, 256:768], in_=HTC[0][:, 256:768])
            nc.sync.dma_start(out=i240[:], in_=I240[:])
            # exp-blocking loads first: the group-0 PE mask tiles (jt 5,7)
            # beat every adjO tile — adjO only feeds the DVE mult, which
            # trails the exps by the e-pool depth.
            nc.sync.dma_start(out=aE0[:, 0:E_CNT[0] * RPC].rearrange(
                "p (k i) -> p k i", k=E_CNT[0]),
                in_=adjE_rows[:, 0:E_CNT[0]])
            nc.sync.dma_start(out=biase[:], in_=BIASE[:])
            nc.sync.dma_start(out=htc[0][:, 768:N // HT_CHUNKS],
                              in_=HTC[0][:, 768:N // HT_CHUNKS])
            for t in (0, 1, 2, 3, 4, 6):
                adj_tile_dma(t)
            for gg in (1, 2):
                if gg == 2:
                    aE[2] = e_group_tile(2)
                    aO[2] = o_group_tile(2)
                nc.sync.dma_start(
                    out=aE[gg][:, 0:E_CNT[gg] * RPC].rearrange(
                        "p (k i) -> p k i", k=E_CNT[gg]),
                    in_=adjE_rows[:, E_BASE[gg]:E_BASE[gg] + E_CNT[gg]])
                nc.sync.dma_start(
                    out=aO[gg][:, 0:O_CNT[gg] * RPC].rearrange(
                        "p (k i) -> p k i", k=O_CNT[gg]),
                    in_=adjO_rows[:, O_BASE[gg]:O_BASE[gg] + O_CNT[gg]])
            for t, src in [(ident, IDENT), (w1v, W1V), (w2, W2),
                           (b1r, B1R), (b2r, B2R), (ones, ONES)]:
                nc.gpsimd.dma_start(out=t[:], in_=src[:])
            nc.gpsimd.dma_start(out=hnc[0][:], in_=HNC[0][:])
            nc.gpsimd.dma_start(out=hnc[1][:], in_=HNC[1][:])

            acc = pers.tile([D, RPC], BF16, tag="acc")
            zsb = pers.tile([D, RPC], F32R, tag="zsb")
            hts = pers.tile([D, RPC], F32R, tag="hts")    # hidden^T (SBUF)
            dentr = pers.tile([1, RPC], F32R, tag="dentr")
            dcol = pers.tile([D, NC], F32, tag="dcol")
            rcol = pers.tile([D, NC], F32, tag="rcol")
            outsb = pers.tile([D, NC * D], F32, tag="outsb")

            # ---- stage 1 (Z matmuls lag two j-tiles so scores(jt+1)
            # issue while exp(jt) runs)
            zps = psZ.tile([D, RPC], F32, tag="z")
            etiles = {}

            def do_z(jt):
                e_prev = etiles.pop(jt)
                htile = hnc[jt // 16][:, (jt % 16) * 128:(jt % 16 + 1) * 128]
                for h in range(2):
                    cs = slice(h * 512, (h + 1) * 512)
                    nc.tensor.matmul(zps[:, cs], lhsT=htile, rhs=e_prev[:, cs],
                                     start=(jt == 0), stop=(jt == JT - 1))

            cwq = N // HT_CHUNKS // 128
            for jt in range(JT):
                g, r = divmod(jt, GRP)
                # per-tile prefetch 2 groups ahead keeps the wires smooth:
                # one adj tile per iteration, H chunks in halves mid-group.
                tgt = jt + 3 * GRP
                if tgt < JT:
                    if tgt % GRP == 0:
                        gg = tgt // GRP
                        aE[gg] = e_group_tile(gg)
                        aO[gg] = o_group_tile(gg)
                    adj_tile_dma(tgt)
                hchalf = N // HT_CHUNKS // 2
                for gc in (1, 2, 3):
                    if jt == 16 * gc - 14:
                        nc.sync.dma_start(out=htc[gc][:, 0:hchalf],
                                          in_=HTC[gc][:, 0:hchalf])
                    elif jt == 16 * gc - 10:
                        nc.sync.dma_start(out=htc[gc][:, hchalf:],
                                          in_=HTC[gc][:, hchalf:])
                for gc in (2, 3):
                    if jt == 16 * gc - 6:
                        nc.sync.dma_start(out=hnc[gc][:], in_=HNC[gc][:])
                even = jt in E_IDX
                sps = psA.tile([D, RPC], F32, tag="big")
                ktile = htc[jt // cwq][:, (jt % cwq) * 128:(jt % cwq + 1) * 128]
                if even:
                    half = E_IDX[jt] - e_base(g)
                    adj_sb = aE[g]
                    for h in range(2):
                        cs = slice(h * 512, (h + 1) * 512)
                        nc.tensor.matmul(sps[:, cs], lhsT=i240[:],
                                         rhs=adj_sb[:, half * RPC + h * 512:
                                                    half * RPC + (h + 1) * 512],
                                         start=True, stop=False)
                        nc.tensor.matmul(sps[:, cs], lhsT=ktile, rhs=qk[:, cs],
                                         start=False, stop=True)
                else:
                    for h in range(2):
                        cs = slice(h * 512, (h + 1) * 512)
                        nc.tensor.matmul(sps[:, cs], lhsT=ktile, rhs=qk[:, cs],
                                         start=True, stop=True)
                e = ep.tile([D, RPC], BF16, tag="e")
                nc.scalar.activation(e[:], sps[:],
                                     mybir.ActivationFunctionType.Exp,
                                     bias=(biase[:] if even else biaso[:]))
                etiles[jt] = e
                if not even:
                    half = O_IDX[jt] - o_base(g)
                    adjo_sb = aO[g]
                    nc.vector.tensor_mul(e[:], e[:],
                                         adjo_sb[:, half * RPC:(half + 1) * RPC])
                if jt == 0:
                    nc.vector.tensor_copy(acc[:], e[:])
                elif jt < JT - 1:
                    nc.vector.tensor_add(acc[:], acc[:], e[:])
                if jt >= 2:
                    do_z(jt - 2)
            do_z(JT - 2)
            e_last = etiles[JT - 1]

            # ---- stage 2: denominators + normalization-deferred transposed MLP
            # den = ones @ (acc + e_last): e(63)'s DVE add is skipped — it
            # rides a second accumulating matmul instead.
            dps = psA.tile([D, RPC], F32, tag="big")
            for h in range(2):
                cs = slice(h * 512, (h + 1) * 512)
                nc.tensor.matmul(dps[:, cs], lhsT=ones[:], rhs=acc[:, cs],
                                 start=True, stop=False)
                nc.tensor.matmul(dps[:, cs], lhsT=ones[:], rhs=e_last[:, cs],
                                 start=False, stop=True)
            do_z(JT - 1)
            nc.vector.tensor_copy(dentr[:, 0:512], dps[0:1, 0:512])
            nc.vector.tensor_copy(dentr[:, 512:1024], dps[0:1, 512:1024])
            nc.scalar.copy(zsb[:, 0:512], zps[:, 0:512])
            nc.vector.tensor_copy(zsb[:, 512:1024], zps[:, 512:1024])
            # 1/denom columns via tiny PE transposes of the den row
            rps = psA.tile([D, RPC], F32, tag="big")
            for it in range(4):
                nc.tensor.transpose(rps[:, it:it + 1],
                                    dentr[0:1, it * 128:(it + 1) * 128].bitcast(F32),
                                    ident[0:1, 0:1])
            gps = psA.tile([D, RPC], F32, tag="big")
            for h in range(2):
                cs = slice(h * 512, (h + 1) * 512)
                nc.tensor.matmul(gps[:, cs], lhsT=w1v[:], rhs=zsb[:, cs],
                                 start=True, stop=not with_bias)
                if with_bias:
                    nc.tensor.matmul(gps[:, cs], lhsT=b1r[:], rhs=dentr[:, cs],
                                     start=False, stop=True)
            for it in range(4, NC):
                nc.tensor.transpose(rps[:, it:it + 1],
                                    dentr[0:1, it * 128:(it + 1) * 128].bitcast(F32),
                                    ident[0:1, 0:1])
            nc.scalar.activation(hts[:, 0:512], gps[:, 0:512],
                                 mybir.ActivationFunctionType.Relu)
            nc.vector.tensor_relu(hts[:, 512:1024], gps[:, 512:1024])
            nc.scalar.copy(dcol[:], rps[:, 0:NC])
            nc.vector.reciprocal(rcol[:], dcol[:])
            # final layer, re-flipped per 128-row block: lhsT = hts block so
            # the output block lands non-transposed in PSUM; relu + (1/d)
            # scale alternates ACT/DVE and writes the staging tile directly.
            outv = OUT.rearrange("(t p) d -> p t d", p=128)
            for it in range(NC):
                bps = psA.tile([D, RPC], F32, tag="big")
                nc.tensor.matmul(bps[:, 0:D], lhsT=hts[:, it * 128:(it + 1) * 128],
                                 rhs=w2[:], start=True, stop=not with_bias)
                if with_bias:
                    nc.tensor.matmul(bps[:, 0:D],
                                     lhsT=dentr[0:1, it * 128:(it + 1) * 128],
                                     rhs=b2r[:], start=False, stop=True)
                ob = outsb[:, it * 128:(it + 1) * 128]
                if it % 2 == 0:
                    nc.scalar.activation(ob, bps[:, 0:D],
                                         mybir.ActivationFunctionType.Relu,
                                         scale=rcol[:, it:it + 1])
                else:
                    nc.vector.tensor_scalar(ob, bps[:, 0:D], rcol[:, it:it + 1],
                                            0.0, op0=mybir.AluOpType.mult,
                                            op1=mybir.AluOpType.max)
                if it % 2 == 1:
                    nc.sync.dma_start(
                        out=outv[:, it - 1:it + 1],
                        in_=outsb[:, (it - 1) * D:(it + 1) * D].rearrange(
                            "p (t d) -> p t d", t=2))
    nc.finalize()
    return nc


def _prep(H, adj, Wq, Wk, Wv, W1, b1, W2, b2):
    f8 = ml_dtypes.float8_e4m3
    bf = ml_dtypes.bfloat16
    H32 = np.asarray(H, dtype=np.float32)
    HT = np.ascontiguousarray(H32.T)
    adj = np.asarray(adj)
    M = (np.asarray(Wq, np.float32) @ np.asarray(Wk, np.float32).T)
    base = {
        "W1V": np.ascontiguousarray(np.asarray(Wv, np.float32) @ np.asarray(W1, np.float32)),
        "W2": np.asarray(W2, np.float32),
        "B1R": np.asarray(b1, np.float32).reshape(1, D),
        "B2R": np.asarray(b2, np.float32).reshape(1, D),
        "I240": (np.eye(D, dtype=np.float32) * MASK_D).astype(f8),
        "ONES": np.ones((D, D), bf),
        "IDENT": np.eye(1, dtype=np.float32),
        "BIASE": np.full((D, 1), -(MASK_D + STAB), np.float32),
        "BIASO": np.full((D, 1), -STAB, np.float32),
    }
    cw = N // HT_CHUNKS
    for t in range(HT_CHUNKS):
        base[f"HT{t}"] = np.ascontiguousarray(HT[:, t * cw:(t + 1) * cw])
    HNP = np.ascontiguousarray(
        H32.reshape(JT, 128, D).transpose(1, 0, 2).reshape(128, N)).astype(bf)
    for t in range(4):
        base[f"HN{t}"] = np.ascontiguousarray(HNP[:, t * (N // 4):(t + 1) * (N // 4)])
    in_maps = []
    for c in range(NC):
        m = dict(base)
        m["QK"] = np.ascontiguousarray(M.T @ HT[:, c * RPC:(c + 1) * RPC])
        adjT4 = np.ascontiguousarray(
            adj[c * RPC:(c + 1) * RPC, :].T).reshape(JT, 128, RPC)
        m["ADJE"] = np.ascontiguousarray(
            adjT4[PE_MASKED].reshape(NE * 128, RPC)).astype(np.float32).astype(f8)
        m["ADJO"] = np.ascontiguousarray(
            adjT4[DVE_MASKED].reshape(NO * 128, RPC)).astype(np.float32).astype(bf)
        in_maps.append(m)
    return in_maps


def kernel(H, adj, Wq, Wk, Wv, W1, b1, W2, b2):
    wb = bool(np.any(np.asarray(b1)) or np.any(np.asarray(b2)))
    key = f"nc{int(wb)}"
    if key not in _CACHED:
        _CACHED[key] = build(with_bias=wb)
    in_maps = _prep(H, adj, Wq, Wk, Wv, W1, b1, W2, b2)
    res = run_bass_kernel_spmd(_CACHED[key], in_maps, list(range(NC)))
    return np.concatenate([res.results[c]["OUT"] for c in range(NC)], axis=0)
